# revision 2
# baseline (speedup 1.0000x reference)
"""Sliding-window GQA attention (Gemma-style) on 8 TRN2 NeuronCores.

Sharding: tensor-parallel over heads. Core c owns q-heads {2c, 2c+1} and
kv-head c. Each core computes Q/K/V projections (+RoPE) for its heads over
the full sequence, then banded sliding-window attention computed in
TRANSPOSED orientation (logits^T [s, q] via K-stationary matmuls) so the
probability tiles feed the PV matmul directly with no PE transposes.
Softmax row-sums come from a PE ones-matmul; normalization is applied to
the PV output via a partition-broadcast reciprocal. A 4-chunk AllToAll
(one per 128-token output tile) reshards the attention output by token so
every core computes the full output projection for its 512-token slice.
Host concatenates slices.

All matmuls run in bf16 (f32 PSUM accumulation); softmax runs in f32.
"""

import os
import sys

for _p in ("/opt/trn_rl_repo",):
    if _p not in sys.path:
        sys.path.insert(0, _p)

import numpy as np
import ml_dtypes

import concourse.bass as bass
import concourse.mybir as mybir
import concourse.tile as tile
from concourse import bacc
from concourse.bass_utils import run_bass_kernel_spmd

F32 = mybir.dt.float32
BF16 = mybir.dt.bfloat16
AF = mybir.ActivationFunctionType
ALU = mybir.AluOpType

B, T, D = 2, 2048, 3584
NQ, NKV, H = 16, 8, 256
SCALAR = 0.0625
SOFT_CAP = 50.0
WINDOW = 1024
ROPE_BASE = 10000.0

NCORES = 8
P = 128
DC = D // P              # 28 contraction chunks
DCH = DC // 2            # 14 chunks per xt half-tile
TQ = T // P              # 16 query tiles per batch
TPIECE = 512             # projection token-piece width
NPIECE = (B * T) // TPIECE
HLOC = 2 * H             # 512 local q-head columns per core
TOK = B * T              # 4096
TPC = TOK // NCORES      # 512 tokens per core after AllToAll
CHTOK = P                # 128 tokens per A2A chunk block
NCH = TPC // CHTOK       # 4 A2A chunks
WTILES = WINDOW // P     # 8
MASKVAL = -1.0e30
NHC = NQ * H // P        # 32 global h chunks
DP = 512                 # output projection d piece

last_result = None       # BassKernelResults of the most recent device run


def _band(i, mode):
    lo = max(0, i - WTILES)
    hi = i if mode == "tril" else min(TQ - 1, i + WTILES)
    return lo, hi


def build(mode):
    assert mode in ("tril", "ones")
    nc = bacc.Bacc("TRN2", target_bir_lowering=False, debug=False,
                   num_devices=NCORES)

    xT = nc.dram_tensor("xT", [D, TOK], BF16, kind="ExternalInput")
    wq = nc.dram_tensor("wq", [D, HLOC], BF16, kind="ExternalInput")
    wk = nc.dram_tensor("wk", [D, H], BF16, kind="ExternalInput")
    wv = nc.dram_tensor("wv", [D, H], BF16, kind="ExternalInput")
    wo = nc.dram_tensor("wo", [NHC, P, D], BF16, kind="ExternalInput")
    ropeq = nc.dram_tensor("ropeq", [2, P, T], BF16, kind="ExternalInput")
    ropek = nc.dram_tensor("ropek", [2, P, T], BF16, kind="ExternalInput")
    out = nc.dram_tensor("out", [TPC, D], F32, kind="ExternalOutput")

    with tile.TileContext(nc) as tc:
        with (
            tc.tile_pool(name="dram", bufs=1, space="DRAM") as dram,
            tc.tile_pool(name="consts", bufs=1) as consts,
            tc.tile_pool(name="qkv", bufs=1) as qkvpool,
        ):
            # A2A bounce buffers: [src_rank_block][local h][tok]
            a2a_in = [dram.tile([NCORES, HLOC, CHTOK], BF16,
                                name=f"a2a_in{m}") for m in range(NCH)]
            a2a_out = [dram.tile([NCORES, HLOC, CHTOK], BF16,
                                 name=f"a2a_out{m}") for m in range(NCH)]

            # ---- constants (masks in [s, q] orientation) ----
            # causal_T: valid (0) where s <= q, MASKVAL where s > q
            causal_T = None
            if mode == "tril":
                causal_T = consts.tile([P, P], F32)
                nc.gpsimd.memset(causal_T, 0.0)
                nc.gpsimd.affine_select(
                    out=causal_T, in_=causal_T, compare_op=ALU.is_ge,
                    fill=MASKVAL, base=0, pattern=[[1, P]],
                    channel_multiplier=-1)
            # upperedge_T (window lower edge, j == i-8): valid (0) where s > q
            upperedge_T = consts.tile([P, P], F32)
            nc.gpsimd.memset(upperedge_T, 0.0)
            nc.gpsimd.affine_select(
                out=upperedge_T, in_=upperedge_T, compare_op=ALU.is_gt,
                fill=MASKVAL, base=0, pattern=[[-1, P]],
                channel_multiplier=1)
            # loweredge_T (window upper edge, j == i+8): valid (0) where s < q
            loweredge_T = None
            if mode == "ones":
                loweredge_T = consts.tile([P, P], F32)
                nc.gpsimd.memset(loweredge_T, 0.0)
                nc.gpsimd.affine_select(
                    out=loweredge_T, in_=loweredge_T, compare_op=ALU.is_gt,
                    fill=MASKVAL, base=0, pattern=[[1, P]],
                    channel_multiplier=-1)
            # ones column for rowsum matmuls
            ones1 = consts.tile([P, 1], BF16)
            nc.gpsimd.memset(ones1, 1.0)

            qts, kts, vsbs = [], [], []
            xT_v = xT[:].rearrange("(c p) t -> p c t", p=P)

            # =================== projections + rope =======================
            with (
                tc.tile_pool(name="weights", bufs=1) as wpool,
                tc.tile_pool(name="xt", bufs=2) as xpool,
                tc.tile_pool(name="rtmp", bufs=1) as rpool,
                tc.tile_pool(name="proj_ps", bufs=4, space="PSUM") as ppsum,
                tc.tile_pool(name="projv_ps", bufs=3, space="PSUM") as vpsum,
            ):
                for b in range(B):
                    qt = qkvpool.tile([P, 4, T], BF16, tag=f"qt{b}",
                                      name=f"qt{b}")
                    kt = qkvpool.tile([P, 2, T], BF16, tag=f"kt{b}",
                                      name=f"kt{b}")
                    vsb = qkvpool.tile([P, TQ, H], BF16, tag=f"v{b}",
                                       name=f"v{b}")
                    qts.append(qt)
                    kts.append(kt)
                    vsbs.append(vsb)

                wq_sb = wpool.tile([P, DC, HLOC], BF16)
                wq_v = wq[:].rearrange("(c p) h -> p c h", p=P)
                wk_sb = wpool.tile([P, DC, H], BF16)
                wv_sb = wpool.tile([P, DC, H], BF16)
                rq_sb = wpool.tile([P, 2, T], BF16)
                rk_sb = wpool.tile([P, 2, T], BF16)

                # first x piece + first weight half land first
                xts = {}
                xts[(0, 0)] = xpool.tile([P, DCH, TPIECE], BF16, tag="xtA", name="xt00")
                nc.sync.dma_start(xts[(0, 0)][:], xT_v[:, :DCH, 0:TPIECE])
                nc.sync.dma_start(wq_sb[:, :DCH, :], wq_v[:, :DCH, :])
                xts[(0, 1)] = xpool.tile([P, DCH, TPIECE], BF16, tag="xtB", name="xt01")
                nc.sync.dma_start(xts[(0, 1)][:], xT_v[:, DCH:, 0:TPIECE])
                nc.sync.dma_start(wq_sb[:, DCH:, :], wq_v[:, DCH:, :])
                nc.sync.dma_start(rq_sb[:], ropeq[:].rearrange("s p t -> p s t"))
                nc.sync.dma_start(wk_sb[:],
                                  wk[:].rearrange("(c p) h -> p c h", p=P))
                nc.sync.dma_start(rk_sb[:], ropek[:].rearrange("s p t -> p s t"))
                nc.sync.dma_start(wv_sb[:],
                                  wv[:].rearrange("(c p) h -> p c h", p=P))

                for pi in range(NPIECE):
                    t0 = pi * TPIECE          # global token offset
                    b = t0 // T
                    tb = t0 % T               # within-batch offset
                    qt, kt, vsb = qts[b], kts[b], vsbs[b]
                    if pi > 0:
                        xts[(pi, 0)] = xpool.tile([P, DCH, TPIECE], BF16,
                                                  tag="xtA", name=f"xt{pi}0")
                        nc.sync.dma_start(xts[(pi, 0)][:],
                                          xT_v[:, :DCH, t0:t0 + TPIECE])
                        xts[(pi, 1)] = xpool.tile([P, DCH, TPIECE], BF16,
                                                  tag="xtB", name=f"xt{pi}1")
                        nc.sync.dma_start(xts[(pi, 1)][:],
                                          xT_v[:, DCH:, t0:t0 + TPIECE])
                    xta, xtb = xts[(pi, 0)], xts[(pi, 1)]

                    def proj_psum_pair(w_sb, hc0):
                        # two h-chunks in lockstep: alternating psum banks
                        # hides each matmul's LDWEIGHTS under the other's
                        # execution (same-bank chains serialize on TRN2).
                        psA = ppsum.tile([P, TPIECE], F32, tag="pq",
                                         name="psA")
                        psB = ppsum.tile([P, TPIECE], F32, tag="pq",
                                         name="psB")
                        for ps, hc in ((psA, hc0), (psB, hc0 + 1)):
                            for dc in range(DC):
                                xsrc = xta if dc < DCH else xtb
                                nc.tensor.matmul(
                                    ps[:], w_sb[:, dc, hc * P:(hc + 1) * P],
                                    xsrc[:, dc % DCH, :],
                                    start=(dc == 0), stop=(dc == DC - 1))
                        return psA, psB

                    def rope(dst, hc0, psA, psB, tab, tb=tb):
                        cos = tab[:, 0, tb:tb + TPIECE]
                        sin = tab[:, 1, tb:tb + TPIECE]
                        t1 = rpool.tile([P, TPIECE], F32, tag="r1")
                        t2 = rpool.tile([P, TPIECE], F32, tag="r2")
                        nc.vector.tensor_tensor(t1[:], psA[:], cos, ALU.mult)
                        nc.vector.tensor_tensor(t2[:], psB[:], sin, ALU.mult)
                        nc.vector.tensor_tensor(
                            dst[:, hc0, tb:tb + TPIECE], t1[:], t2[:],
                            ALU.subtract)
                        t3 = rpool.tile([P, TPIECE], F32, tag="r3")
                        t4 = rpool.tile([P, TPIECE], F32, tag="r4")
                        nc.vector.tensor_tensor(t3[:], psB[:], cos, ALU.mult)
                        nc.vector.tensor_tensor(t4[:], psA[:], sin, ALU.mult)
                        nc.vector.tensor_tensor(
                            dst[:, hc0 + 1, tb:tb + TPIECE], t3[:], t4[:],
                            ALU.add)

                    for hh in range(2):
                        psA, psB = proj_psum_pair(wq_sb, hh * 2)
                        rope(qt, hh * 2, psA, psB, rq_sb)
                    psA, psB = proj_psum_pair(wk_sb, 0)
                    rope(kt, 0, psA, psB, rk_sb)
                    # V: natural [tok, h] layout; token-subtile pairs in
                    # lockstep for the same LDW-hiding reason.
                    for sub in range(TPIECE // P):
                        pv = vpsum.tile([P, H], F32, tag="pv", name="pv")
                        for dc in range(DC):
                            xsrc = xta if dc < DCH else xtb
                            nc.tensor.matmul(
                                pv[:],
                                xsrc[:, dc % DCH, sub * P:(sub + 1) * P],
                                wv_sb[:, dc, :],
                                start=(dc == 0), stop=(dc == DC - 1))
                        nc.vector.tensor_copy(
                            out=vsb[:, (tb // P) + sub, :], in_=pv[:])

            # ============ banded attention (transposed) + A2A =============
            with (
                tc.tile_pool(name="attn_sb", bufs=4) as apool,
                tc.tile_pool(name="exp_sb", bufs=2) as epool,
                tc.tile_pool(name="stat_sb", bufs=4) as spool,
                tc.tile_pool(name="qk_ps", bufs=3, space="PSUM") as qkpsum,
                tc.tile_pool(name="enc_ps", bufs=2, space="PSUM") as encpsum,
                tc.tile_pool(name="rs_ps", bufs=1, space="PSUM") as rspsum,
                tc.tile_pool(name="oproj", bufs=2) as opool,
                tc.tile_pool(name="enc_full", bufs=1) as efpool,
                tc.tile_pool(name="oproj_ps", bufs=2, space="PSUM") as opsum,
            ):
                def attend(b, i):
                    qt, kt, vsb = qts[b], kts[b], vsbs[b]
                    qt_v = qt[:].rearrange("p (hh c) t -> p c hh t", c=2)
                    lo, hi = _band(i, mode)
                    nj = hi - lo + 1
                    ets = []
                    for j in range(lo, hi + 1):
                        qk = qkpsum.tile([P, 2, P], F32, tag="qk")
                        for c in range(2):
                            nc.tensor.matmul(
                                qk[:], kt[:, c, j * P:(j + 1) * P],
                                qt_v[:, c, :, i * P:(i + 1) * P],
                                start=(c == 0), stop=(c == 1))
                        ts_ = spool.tile([P, 2, P], F32, tag="ts")
                        nc.scalar.activation(ts_[:], qk[:], AF.Tanh,
                                             scale=1.0 / SOFT_CAP)
                        m = None
                        if j == i - WTILES:
                            m = upperedge_T
                        elif j == i and mode == "tril":
                            m = causal_T
                        elif j == i + WTILES and mode == "ones":
                            m = loweredge_T
                        if m is not None:
                            for hh in range(2):
                                nc.vector.tensor_tensor(
                                    ts_[:, hh, :], ts_[:, hh, :], m[:],
                                    ALU.add)
                        et = epool.tile([P, 2, P], BF16, tag=f"et{j - lo}")
                        nc.scalar.activation(et[:], ts_[:], AF.Exp,
                                             scale=SOFT_CAP)
                        ets.append(et)
                    # PV + softmax denominators. Accumulation chains into
                    # the SAME psum bank must stay contiguous (interleaving
                    # two open groups within a bank corrupts accumulation);
                    # chains in different banks may interleave, which hides
                    # LDWEIGHTS. enc c=0/c=1 share one bank, rs has its own.
                    rs = rspsum.tile([1, 2, P], F32, tag="rs")
                    enc = encpsum.tile([P, 2, 2, P], F32, tag="enc")
                    for jj, et in enumerate(ets):
                        nc.tensor.matmul(rs[:], ones1[:], et[:],
                                         start=(jj == 0), stop=(jj == nj - 1))
                    for c in range(2):
                        for jj, et in enumerate(ets):
                            nc.tensor.matmul(
                                enc[:, c, :, :],
                                vsb[:, lo + jj, c * P:(c + 1) * P], et[:],
                                start=(jj == 0), stop=(jj == nj - 1))
                    rinv = spool.tile([1, 2, P], F32, tag="rinv")
                    nc.vector.reciprocal(rinv[:], rs[:])
                    rb = spool.tile([P, 2, P], F32, tag="rb")
                    nc.gpsimd.partition_broadcast(rb[:], rinv[:])
                    encsb = apool.tile([P, 2, 2, P], BF16, tag="encsb")
                    for c in range(2):
                        nc.vector.tensor_tensor(
                            encsb[:, :, c, :], enc[:, c, :, :], rb[:],
                            ALU.mult)
                    gtok = b * T + i * P
                    jb = gtok // TPC
                    m4 = i % NCH
                    dst = a2a_in[m4][jb].rearrange("(c4 p) t -> p c4 t", p=P)
                    nc.sync.dma_start(
                        dst, encsb[:].rearrange("p hh c q -> p (hh c) q"))

                wo_sb = {}

                def prefetch_wo(dp):
                    wo_sb[dp] = opool.tile([P, NHC, DP], BF16, tag="wo",
                                            name=f"wo{dp}")
                    nc.sync.dma_start(
                        wo_sb[dp][:],
                        wo[:, :, dp * DP:(dp + 1) * DP].rearrange(
                            "c p d -> p c d"))

                for m in range(NCH):
                    if m == 1:
                        prefetch_wo(0)
                    if m == NCH - 1:
                        prefetch_wo(1)
                    for b in range(B):
                        for i in range(TQ):
                            if i % NCH == m:
                                attend(b, i)
                    if m == NCH - 1:
                        # Barrier BEFORE the last collective: the barrier's
                        # backward sync edges wait for collective
                        # completions, so placing it after A2A3 would stall
                        # oproj start until the last exchange finishes.
                        tc.strict_bb_all_engine_barrier()
                    nc.gpsimd.collective_compute(
                        "AllToAll", ALU.bypass,
                        replica_groups=[list(range(NCORES))],
                        ins=[a2a_in[m][:].opt()],
                        outs=[a2a_out[m][:].opt()])

                # =================== output projection ====================
                efs = []
                for m in range(NCH):
                    efc = efpool.tile([P, NHC, CHTOK], BF16, tag=f"ef{m}")
                    if m < NCH - 1:
                        nc.sync.dma_start(
                            efc[:],
                            a2a_out[m][:].rearrange("b (c p) t -> p (b c) t",
                                                    p=P))
                    efs.append(efc)

                def po_group(dp, m):
                    po = opsum.tile([P, DP], F32, tag="po", name="po")
                    for hc in range(NHC):
                        nc.tensor.matmul(
                            po[:], efs[m][:, hc, :], wo_sb[dp][:, hc, :],
                            start=(hc == 0), stop=(hc == NHC - 1))
                    osb = opool.tile([P, DP], F32, tag="osb", name="osb")
                    nc.vector.tensor_copy(out=osb[:], in_=po[:])
                    nc.sync.dma_start(
                        out[m * P:(m + 1) * P, dp * DP:(dp + 1) * DP],
                        osb[:])

                # dp0/dp1 run their m0-m2 groups first so the ef3 load
                # (gated on the last collective) has time to land; their m3
                # groups are deferred to the end of dp1.
                plan = [(0, 0), (0, 1), (0, 2), (1, 0), (1, 1), (0, 3),
                        (1, 2), (1, 3)]
                plan += [(dp, m) for dp in range(2, D // DP)
                         for m in range(NCH)]
                for k, (dp, m) in enumerate(plan):
                    if dp not in wo_sb:
                        prefetch_wo(dp)
                    po_group(dp, m)
                    if k == 0:
                        # ef3 load emitted late: its wait on the last
                        # collective must not gate the ef0-2 consumers
                        # (shared DMA-completion semaphore thresholds
                        # are cumulative).
                        nc.sync.dma_start(
                            efs[NCH - 1][:],
                            a2a_out[NCH - 1][:].rearrange(
                                "b (c p) t -> p (b c) t", p=P))

    nc.compile()
    return nc


def _rope_tables(pos, scale):
    """pos: [T] int array -> [2, 128, T] bf16 (cos;sin), scaled."""
    frac = 2.0 * np.arange(H // 2, dtype=np.float64) / H
    timescale = ROPE_BASE ** frac                      # [128]
    sinusoid = pos.astype(np.float64)[None, :] / timescale[:, None]  # [128,T]
    tabs = np.stack([np.cos(sinusoid), np.sin(sinusoid)]) * scale
    return tabs.astype(ml_dtypes.bfloat16)


def _reference_host(x, segment_pos, attn_mask, w_q, w_kv, w_o):
    """Slow but fully general fallback (numpy)."""
    xb = x.astype(np.float32)
    q = np.einsum('btd,ndh->btnh', xb, w_q)
    k = np.einsum('bsd,kdh->bskh', xb, w_kv[0])
    v = np.einsum('bsd,kdh->bskh', xb, w_kv[1])

    def rope(t, positions):
        hd = t.shape[-1]
        frac = 2.0 * np.arange(hd // 2, dtype=np.float32) / hd
        ts_ = ROPE_BASE ** frac
        sinusoid = positions.astype(np.float32)[..., None] / ts_
        sinusoid = sinusoid[..., None, :]
        s, c = np.sin(sinusoid), np.cos(sinusoid)
        first, second = np.split(t, 2, axis=-1)
        return np.concatenate([first * c - second * s,
                               second * c + first * s], axis=-1)

    q = rope(q, segment_pos) * SCALAR
    k = rope(k, segment_pos)
    qg = q.reshape(B, T, NKV, 2, H)
    logits = np.einsum('btkgh,bskh->btkgs', qg, k).reshape(B, T, NQ, T)
    logits = np.tanh(logits / SOFT_CAP) * SOFT_CAP
    pos_s = np.arange(T)[None, None, :]
    pos_t = segment_pos[:, :, None]
    sliding = (pos_s > pos_t - WINDOW) & (pos_s < pos_t + WINDOW)
    mask = np.logical_and(attn_mask, sliding)
    padded = np.where(mask[:, :, None, :], logits, -np.inf)
    padded -= padded.max(axis=-1, keepdims=True)
    e = np.exp(padded)
    probs = (e / e.sum(axis=-1, keepdims=True)).astype(np.float32)
    v_exp = np.repeat(v, NQ // NKV, axis=2)            # [B,T,NQ,H]
    enc = np.einsum('btns,bsnh->btnh', probs, v_exp)
    return np.einsum('btnh,nhd->btd', enc, w_o).astype(np.float32)


_GRAPH_CACHE = {}


def kernel(x, segment_pos, attn_mask, w_q, w_kv, w_o):
    global last_result
    x = np.asarray(x)
    segment_pos = np.asarray(segment_pos)
    attn_mask = np.asarray(attn_mask)
    w_q = np.asarray(w_q, dtype=np.float32)
    w_kv = np.asarray(w_kv, dtype=np.float32)
    w_o = np.asarray(w_o, dtype=np.float32)

    arange = np.broadcast_to(np.arange(T, dtype=segment_pos.dtype), (B, T))
    std_pos = np.array_equal(segment_pos, arange)
    tril = np.broadcast_to(np.tril(np.ones((T, T), dtype=bool)), (B, T, T))
    if attn_mask.all():
        mode = "ones"
    elif np.array_equal(attn_mask, tril):
        mode = "tril"
    else:
        mode = None
    if not std_pos or mode is None:
        return _reference_host(x, segment_pos, attn_mask, w_q, w_kv, w_o)

    if mode not in _GRAPH_CACHE:
        _GRAPH_CACHE[mode] = build(mode)
    nc = _GRAPH_CACHE[mode]

    bf = ml_dtypes.bfloat16
    xT = np.ascontiguousarray(x.reshape(TOK, D).T).astype(bf)    # [D, TOK]
    pos = segment_pos[0]
    ropeq = np.ascontiguousarray(_rope_tables(pos, SCALAR))
    ropek = np.ascontiguousarray(_rope_tables(pos, 1.0))
    wo_all = np.ascontiguousarray(
        w_o.reshape(NHC, P, D)).astype(bf)

    in_maps = []
    for c in range(NCORES):
        wq_c = np.ascontiguousarray(
            np.concatenate([w_q[2 * c], w_q[2 * c + 1]], axis=1)).astype(bf)
        wk_c = np.ascontiguousarray(w_kv[0, c]).astype(bf)
        wv_c = np.ascontiguousarray(w_kv[1, c]).astype(bf)
        in_maps.append({
            "xT": xT, "wq": wq_c, "wk": wk_c, "wv": wv_c, "wo": wo_all,
            "ropeq": ropeq, "ropek": ropek,
        })

    trace = os.environ.get("KTRACE", "0") == "1"
    res = run_bass_kernel_spmd(nc, in_maps, core_ids=list(range(NCORES)),
                               trace=trace)
    last_result = res
    outs = [res.results[c]["out"] for c in range(NCORES)]
    return np.concatenate(outs, axis=0).reshape(B, T, D).astype(np.float32)
